# revision 3
# baseline (speedup 1.0000x reference)
"""Multi-head causal attention (B=4, T=2048, D=1024, H=16) on 8 NeuronCores.

Sharding: data-parallel over batch (4) x tensor-parallel over heads (2 groups
of 8 heads) = 8 cores. Each core runs the QKV projection for its head group,
causal flash-attention, then an AllGather of head outputs within each batch
pair and a column-sharded output projection — the head-mixing reduction
happens on device; the host only concatenates disjoint output column slices.

All matmuls run in float32r (TF32-like: full rate on TRN2 for free dim >= 256,
~1e-4 relative error). Scores are computed transposed, S^T[s, tq], so the
softmax normalizer comes free from a ones-column appended to V, and the
1/sqrt(dh) score scale folds into the ACT exp's scale argument. Only
lower-triangle score blocks are computed; diagonal blocks get an additive
triangular mask and a narrowed matmul/exp width.
"""
import sys

sys.path.insert(0, '/opt/trn_rl_repo')

import numpy as np

import concourse.mybir as mybir
import concourse.tile as tile
from concourse import bacc
from concourse.bass_utils import run_bass_kernel_spmd

B, T, D = 4, 2048, 1024
H, DH = 16, 64
HG = 8              # heads per core
GD = HG * DH        # 512 features per core
P = 128
CH = 512            # tq chunk width (one psum bank)
NB = T // P         # 16 s-blocks
NCH = T // CH       # 4 tq chunks
KB = D // P         # 8 contraction blocks over d_model
F32 = mybir.dt.float32
F32R = mybir.dt.float32r

_CACHE = {}


def build_nc():
    nc = bacc.Bacc("TRN2", target_bir_lowering=False, debug=False)

    xT = nc.dram_tensor("xT", [D, T], F32R, kind="ExternalInput")
    wq = nc.dram_tensor("wq", [D, GD], F32R, kind="ExternalInput")
    wk = nc.dram_tensor("wk", [D, GD], F32R, kind="ExternalInput")
    wv = nc.dram_tensor("wv", [D, GD], F32R, kind="ExternalInput")
    wout = nc.dram_tensor("wout", [D, GD], F32R, kind="ExternalInput")
    tril = nc.dram_tensor("tril", [P, P], F32, kind="ExternalInput")
    onecol = nc.dram_tensor("onecol", [P, 1], F32R, kind="ExternalInput")
    y = nc.dram_tensor("y", [T, GD], F32, kind="ExternalOutput")

    ot_dram = nc.dram_tensor("ot_cc_in", [GD, T], F32R)         # own O^T
    otf_dram = nc.dram_tensor("otf_cc_out", [2 * GD, T], F32R)  # pair-gathered

    with tile.TileContext(nc) as tc:
        with tc.tile_pool(name="big", bufs=1) as big, \
             tc.tile_pool(name="wres", bufs=1) as wres, \
             tc.tile_pool(name="cst", bufs=1) as cst, \
             tc.tile_pool(name="wstr", bufs=2) as wstr, \
             tc.tile_pool(name="qk", bufs=1) as qkp, \
             tc.tile_pool(name="vp", bufs=1) as vput, \
             tc.tile_pool(name="exps", bufs=3) as expp, \
             tc.tile_pool(name="sml", bufs=4) as sml, \
             tc.tile_pool(name="ystg", bufs=3) as ystg, \
             tc.tile_pool(name="ps_a", bufs=2, space="PSUM") as ps_a, \
             tc.tile_pool(name="ps_s", bufs=3, space="PSUM") as ps_s, \
             tc.tile_pool(name="ps_o", bufs=2, space="PSUM") as ps_o:

            # ---- resident loads ----
            xT_sb = big.tile([P, KB, T], F32R, tag="big")
            nc.sync.dma_start(xT_sb[:], xT.ap().rearrange("(ko p) t -> p ko t", p=P))
            wv_sb = wres.tile([P, KB, GD], F32R, tag="wv")
            nc.sync.dma_start(wv_sb[:], wv.ap().rearrange("(ko p) n -> p ko n", p=P))
            wo_sb = wres.tile([P, KB, GD], F32R, tag="wo")
            nc.sync.dma_start(wo_sb[:], wout.ap().rearrange("(ko p) n -> p ko n", p=P))
            tril_sb = cst.tile([P, P], F32)
            nc.sync.dma_start(tril_sb[:], tril.ap())
            one_sb = cst.tile([P, 1], F32R)
            nc.sync.dma_start(one_sb[:], onecol.ap())

            # ---- V projection; V_aug[:, nb, h, 0:64] = v, [.., 64] = 1 ----
            v_aug = vput.tile([P, NB, HG, DH + 1], F32R)
            nc.vector.tensor_copy(
                out=v_aug[:, :, :, DH:DH + 1],
                in_=one_sb[:, :, None, None].to_broadcast((P, NB, HG, 1)),
            )
            for sb_i in range(NB):
                psum = ps_a.tile([P, CH], F32, tag="proj")
                for k in range(KB):
                    nc.tensor.matmul(
                        psum[:],
                        lhsT=xT_sb[:, k, sb_i * P:(sb_i + 1) * P],
                        rhs=wv_sb[:, k, :],
                        start=(k == 0), stop=(k == KB - 1),
                    )
                for h in range(HG):
                    nc.vector.tensor_copy(
                        out=v_aug[:, sb_i, h, 0:DH],
                        in_=psum[:, h * DH:(h + 1) * DH],
                    )

            # ---- per head-pair: project Q^T/K^T for 2 heads, then attend ----
            for hh in range(HG // 2):
                wq_sb = wstr.tile([P, KB, P], F32R, tag="wq")
                wk_sb = wstr.tile([P, KB, P], F32R, tag="wk")
                nc.sync.dma_start(
                    wq_sb[:],
                    wq.ap()[:, hh * P:(hh + 1) * P].rearrange("(ko p) n -> p ko n", p=P))
                nc.sync.dma_start(
                    wk_sb[:],
                    wk.ap()[:, hh * P:(hh + 1) * P].rearrange("(ko p) n -> p ko n", p=P))
                qt = qkp.tile([P, T], F32R, tag="qt")
                kt = qkp.tile([P, T], F32R, tag="kt")
                for c4 in range(T // CH):
                    for (w_sb, dst) in ((wq_sb, qt), (wk_sb, kt)):
                        psum = ps_a.tile([P, CH], F32, tag="proj")
                        for k in range(KB):
                            nc.tensor.matmul(
                                psum[:],
                                lhsT=w_sb[:, k, :],
                                rhs=xT_sb[:, k, c4 * CH:(c4 + 1) * CH],
                                start=(k == 0), stop=(k == KB - 1),
                            )
                        nc.vector.tensor_copy(
                            out=dst[:, c4 * CH:(c4 + 1) * CH], in_=psum[:])

                for h2 in range(2):
                    h = hh * 2 + h2
                    pb = h2 * DH   # partition base of this head in qt/kt
                    for c in range(NCH):
                        ot_ps = ps_o.tile([DH + 1, CH], F32, tag="ot")
                        nblk = (c + 1) * (CH // P)
                        for i in range(nblk):
                            r = i - c * (CH // P)
                            if r >= 0:          # diagonal-region block
                                f0 = P * r
                                w = max(CH - f0, P)
                            else:
                                f0, w = 0, CH
                            s_ps = ps_s.tile([P, CH], F32, tag="s")
                            nc.tensor.matmul(
                                s_ps[:, f0:f0 + w],
                                lhsT=kt[pb:pb + DH, i * P:(i + 1) * P],
                                rhs=qt[pb:pb + DH, c * CH + f0:c * CH + f0 + w],
                                start=True, stop=True,
                            )
                            if r >= 0:
                                nc.vector.tensor_add(
                                    out=s_ps[:, f0:f0 + P],
                                    in0=s_ps[:, f0:f0 + P], in1=tril_sb[:])
                            e_sb = expp.tile([P, CH], F32R, tag="e")
                            nc.scalar.activation(
                                e_sb[:, f0:f0 + w], s_ps[:, f0:f0 + w],
                                mybir.ActivationFunctionType.Exp,
                                scale=float(DH ** -0.5))
                            nc.tensor.matmul(
                                ot_ps[:, f0:f0 + w],
                                lhsT=v_aug[:, i, h, :],
                                rhs=e_sb[:, f0:f0 + w],
                                start=(i == 0), stop=(i == nblk - 1),
                            )
                        recip = sml.tile([1, CH], F32, tag="recip")
                        nc.vector.reciprocal(recip[:], ot_ps[DH:DH + 1, :])
                        bcast = sml.tile([DH, CH], F32, tag="bcast")
                        nc.gpsimd.partition_broadcast(bcast[:], recip[:])
                        ot_sb = sml.tile([DH, CH], F32R, tag="otsb")
                        nc.vector.tensor_mul(
                            out=ot_sb[:], in0=ot_ps[0:DH, :], in1=bcast[:])
                        nc.sync.dma_start(
                            ot_dram.ap()[h * DH:(h + 1) * DH,
                                         c * CH:(c + 1) * CH],
                            ot_sb[:])

            # ---- AllGather O^T within batch pairs ----
            nc.gpsimd.collective_compute(
                "AllGather",
                mybir.AluOpType.bypass,
                replica_groups=[[0, 1], [2, 3], [4, 5], [6, 7]],
                ins=[ot_dram.ap()],
                outs=[otf_dram.ap()],
            )

            # ---- output projection: y[tq, :] = OTF.T @ wout_cols ----
            otf_sb = big.tile([P, 2 * GD // P, T], F32R, tag="big")
            nc.sync.dma_start(
                otf_sb[:], otf_dram.ap().rearrange("(ko p) t -> p ko t", p=P))
            for m in range(T // P):
                psum = ps_a.tile([P, CH], F32, tag="proj")
                for k in range(KB):
                    nc.tensor.matmul(
                        psum[:],
                        lhsT=otf_sb[:, k, m * P:(m + 1) * P],
                        rhs=wo_sb[:, k, :],
                        start=(k == 0), stop=(k == KB - 1),
                    )
                y_sb = ystg.tile([P, GD], F32)
                nc.vector.tensor_copy(out=y_sb[:], in_=psum[:])
                nc.sync.dma_start(y.ap()[m * P:(m + 1) * P, :], y_sb[:])

    nc.compile()
    return nc


def _get_nc():
    if 'nc' not in _CACHE:
        _CACHE['nc'] = build_nc()
    return _CACHE['nc']


def _make_in_maps(inputs):
    x = np.asarray(inputs["x"], dtype=np.float32)
    W_qkv = np.asarray(inputs["W_qkv"], dtype=np.float32)
    W_out = np.asarray(inputs["W_out"], dtype=np.float32)

    tril_m = np.where(
        np.arange(P)[:, None] <= np.arange(P)[None, :], 0.0, -1e30
    ).astype(np.float32)
    ones = np.ones((P, 1), np.float32)

    in_maps = []
    for core in range(8):
        b, g = core // 2, core % 2
        in_maps.append({
            "xT": np.ascontiguousarray(x[b].T),
            "wq": np.ascontiguousarray(W_qkv[:, g * GD:(g + 1) * GD]),
            "wk": np.ascontiguousarray(W_qkv[:, D + g * GD:D + (g + 1) * GD]),
            "wv": np.ascontiguousarray(W_qkv[:, 2 * D + g * GD:2 * D + (g + 1) * GD]),
            "wout": np.ascontiguousarray(W_out[:, g * GD:(g + 1) * GD]),
            "tril": tril_m,
            "onecol": ones,
        })
    return in_maps


def kernel(x, W_qkv, W_out, mask):
    """Full inputs in, full output out. mask is the known causal tril."""
    in_maps = _make_in_maps({"x": x, "W_qkv": W_qkv, "W_out": W_out})
    nc = _get_nc()
    res = run_bass_kernel_spmd(nc, in_maps, core_ids=list(range(8)))

    out = np.empty((B, T, D), dtype=np.float32)
    for core in range(8):
        b, g = core // 2, core % 2
        out[b, :, g * GD:(g + 1) * GD] = res.results[core]["y"]
    return out
